# revision 1
# baseline (speedup 1.0000x reference)
"""LSTM (B=4096, S=512, I=1, H=50) Bass kernel for 8 TRN2 NeuronCores.

Strategy: data-parallel over batch (512 rows per core). Per core the scan
runs with hidden on SBUF partitions and batch on the free dim, so h comes
out of the elementwise stage already transposed for the next matmul.

Math tricks (all host-side weight preprocessing):
  - sigmoid(x) = (1 + tanh(x/2)) / 2  -> every gate is a single Tanh; all
    four gates of one step live in 2 ACT instructions.
  - State D = 2c and H = 2h absorb the /2 factors:
        D' = 0.5*(1+tf)*D + (1+ti)*tg        (3 scalar_tensor_tensor ops)
        H' = (1+to) * tanh(0.5*D')           (1 ACT + 1 STT op)
    with W_hh pre-scaled by 0.5 column-wise (H=2h input) and gate rows
    scaled 0.5 (i,f,o) / 1.0 (g).
  - x-projection and bias folded into the recurrence matmul by augmenting
    the state tile with an x-row and a ones-row (K = 50+2 = 52).

Batch is split in two groups of 256 per core so the two dependency chains
pipeline across engines.
"""

import numpy as np

B, S, H = 4096, 512, 50
NCORES = 8
BS = B // NCORES          # 512 batch rows per core
G = 2                     # pipeline groups per core
GN = BS // G              # 256 batch columns per group
KK = 114                  # rows: 0=x, 1=ones, 64:114 = H-state
RB = 32                   # ring slots / x-staging block

_cache = {}


def _build(b_fc_val: float):
    import concourse.bass as bass
    import concourse.mybir as mybir
    from concourse.tile import TileContext
    from concourse.vector_clock import ScopedClock

    class TC1W(TileContext):
        # this walrus accepts only ONE sem wait per instruction; split any
        # instruction's extra waits onto preceding same-engine NOPs
        def _split_multiwaits(self):
            nc_ = self.nc
            cnt = 0
            for f_ in nc_.m.functions:
                for bb in f_.blocks:
                    il = list(bb.instructions)
                    out, changed = [], False
                    for ins in il:
                        si = ins.sync_info
                        if si is not None and si.on_wait and len(si.on_wait) > 1:
                            waits = list(si.on_wait)
                            for w in waits[:-1]:
                                cnt += 1
                                nop = mybir.InstNoOp(
                                    name=f"wsplit{cnt}", ins=[], outs=[])
                                nop.engine = ins.engine
                                nop.sync_info = mybir.SyncInfo(
                                    on_wait=[w], on_update=[])
                                out.append(nop)
                            si.on_wait = waits[-1:]
                            changed = True
                        out.append(ins)
                    if changed:
                        bb.instructions = out

        def _drain_and_barrier(self, tick_clock, wait_clock):
            nc_ = self.nc
            self._split_multiwaits()
            drain_inst = nc_.sync.drain()
            wait_clock.add_sem_waits(
                drain_inst.ins, ScopedClock({None: tick_clock.global_clock}))
            si = drain_inst.ins.sync_info
            waits = list(si.on_wait) if si is not None and si.on_wait else []
            if len(waits) > 1:
                si.on_wait = waits[:1]
                for w in waits[1:]:
                    d2 = nc_.sync.drain()
                    si2 = d2.ins.sync_info
                    if si2 is None:
                        d2.ins.sync_info = mybir.SyncInfo(on_wait=[w],
                                                          on_update=[])
                    else:
                        si2.on_wait = [w]
            nc_.all_engine_barrier()
            popped = nc_._tile_sem_poison_stack.pop()
            assert popped is self._sem_poison
            nc_.clear_and_free_semaphores(list(self.sems.allocated().values()))
            nc_.all_engine_barrier()

    fp32 = mybir.dt.float32
    Tanh = mybir.ActivationFunctionType.Tanh
    add = mybir.AluOpType.add
    mult = mybir.AluOpType.mult

    nc = bass.Bass("TRN2")

    xT = nc.dram_tensor("xT", [S, BS], fp32, kind="ExternalInput")
    w_ifb = nc.dram_tensor("w_ifb", [KK, 128], fp32, kind="ExternalInput")
    w_gob = nc.dram_tensor("w_gob", [KK, 128], fp32, kind="ExternalInput")
    w_fc = nc.dram_tensor("w_fc", [KK, 1], fp32, kind="ExternalInput")
    wx_if_d = nc.dram_tensor("wx_if", [1, 128], fp32, kind="ExternalInput")
    wx_go_d = nc.dram_tensor("wx_go", [1, 128], fp32, kind="ExternalInput")
    out_d = nc.dram_tensor("out", [1, BS], fp32, kind="ExternalOutput")

    xT3 = xT.rearrange("(o s) b -> o s b", o=1)

    with TC1W(nc) as tc:
        with (
            tc.tile_pool(name="const", bufs=1) as cpool,
            tc.tile_pool(name="work", bufs=2) as wpool,
            tc.tile_pool(name="psum", bufs=2, space="PSUM") as ppool,
        ):
            w_ifb_sb = cpool.tile([KK, 128], fp32, tag="w_ifb")
            w_gob_sb = cpool.tile([KK, 128], fp32, tag="w_gob")
            w_fc_sb = cpool.tile([KK, 1], fp32, tag="w_fc")
            wx_if_sb = cpool.tile([1, 128], fp32, tag="wx_if")
            wx_go_sb = cpool.tile([1, 128], fp32, tag="wx_go")
            nc.gpsimd.dma_start(wx_if_sb[:], wx_if_d[:])
            nc.gpsimd.dma_start(wx_go_sb[:], wx_go_d[:])
            nc.gpsimd.dma_start(w_ifb_sb[:], w_ifb[:])
            nc.gpsimd.dma_start(w_gob_sb[:], w_gob[:])
            nc.gpsimd.dma_start(w_fc_sb[:], w_fc[:])

            # ring tiles: row 0 = x, row 1 = ones, rows HB:HB+H = H-state(=2h)
            # 64 slots of GN columns each
            RT = [cpool.tile([KK, RB * GN], fp32, tag=f"RT{g}", name=f"RT{g}")
                  for g in range(G)]
            Dst = [cpool.tile([128, GN], fp32, tag=f"D{g}", name=f"D{g}")
                   for g in range(G)]
            XR = [cpool.tile([1, RB * GN], fp32, tag=f"XR{g}", name=f"XR{g}")
                  for g in range(G)]
            jnk = [cpool.tile([1, 1], fp32, tag=f"jnk{g}", name=f"jnk{g}")
                   for g in range(G)]
            for g in range(G):
                nc.vector.memset(RT[g][:], 0.0)
                nc.vector.memset(RT[g][0:1, :], 1.0)
                nc.vector.memset(Dst[g][:], 0.0)
                # x block 0: slots 0..RB-1
                nc.gpsimd.dma_start(
                    XR[g][0:1, :].rearrange("o (a b) -> o a b", b=GN),
                    xT3[0:1, 0:RB, g * GN : (g + 1) * GN])

            # wait-carrier dummies: absorb one DMA sem each on the PE
            pcar = ppool.tile([128, GN], fp32, tag="zA0", name="pcar")
            for src in (w_ifb_sb, w_gob_sb, w_fc_sb, wx_if_sb, wx_go_sb,
                        XR[0], XR[1]):
                nc.tensor.matmul(pcar[0:1, 0:1], src[0:1, 0:1],
                                 src[0:1, 0:1], skip_group_check=True)

            TAhist = {0: [], 1: []}
            for t in range(S):
                sl = t % RB
                sn = (t + 1) % RB
                for g in range(G):
                    cols = slice(sl * GN, (sl + 1) * GN)
                    ncols = slice(sn * GN, (sn + 1) * GN)
                    # stage next x block (one DMA per RB steps)
                    if t % RB == 0 and t + RB < S:
                        nc.gpsimd.dma_start(
                            XR[g][0:1, :].rearrange("o (a b) -> o a b", b=GN),
                            xT3[0:1, t + RB : t + 2 * RB,
                                g * GN : (g + 1) * GN])

                    zA = ppool.tile([128, GN], fp32, tag=f"zA{g}")
                    zB = ppool.tile([128, GN], fp32, tag=f"zB{g}")
                    if len(TAhist[g]) >= 2:
                        # PE carrier: absorb the ACT tick (zA/zB slot WAR)
                        ta_old = TAhist[g][-2]
                        nc.tensor.matmul(zA[0:1, 0:1], ta_old[0:1, 0:1],
                                         ta_old[0:1, 0:1],
                                         skip_group_check=True)
                    nc.tensor.matmul(zA[:], w_ifb_sb[:], RT[g][:, cols],
                                     start=True, stop=False)
                    nc.tensor.matmul(zA[:], wx_if_sb[:], XR[g][0:1, cols],
                                     start=False, stop=True)
                    nc.tensor.matmul(zB[:], w_gob_sb[:], RT[g][:, cols],
                                     start=True, stop=False)
                    nc.tensor.matmul(zB[:], wx_go_sb[:], XR[g][0:1, cols],
                                     start=False, stop=True)

                    # all-tanh gates: TA = [ti @0 ; tf @64], TB = [tg @0 ; to @64]
                    TA = wpool.tile([128, GN], fp32, tag=f"TA{g}")
                    TB = wpool.tile([128, GN], fp32, tag=f"TB{g}")
                    nc.scalar.activation(TA[:], zA[:], Tanh)
                    nc.scalar.activation(TB[:], zB[:], Tanh)
                    TAhist[g].append(TA)
                    # DVE carrier: absorb the PE tick (covers ring WAR for H2)
                    nc.vector.tensor_copy(jnk[g][0:1, 0:1], zB[0:1, 0:1])

                    # D' = 0.5*(1+tf)*D + (1+ti)*tg      (state D = 2c @64)
                    Bt = wpool.tile([H, GN], fp32, tag=f"Bt{g}")
                    At = wpool.tile([H, GN], fp32, tag=f"At{g}")
                    nc.vector.scalar_tensor_tensor(
                        Bt[:], TA[64 : 64 + H, :], 1.0,
                        Dst[g][64 : 64 + H, :], add, mult)
                    nc.vector.scalar_tensor_tensor(
                        At[:], TA[0:H, :], 1.0, TB[0:H, :], add, mult)
                    nc.vector.scalar_tensor_tensor(
                        Dst[g][64 : 64 + H, :], Bt[:], 0.5, At[:], mult, add)

                    # H' = (1+to) * tanh(0.5*D') -> ring slot t+1, rows 64:114
                    TD = wpool.tile([128, GN], fp32, tag=f"TD{g}")
                    nc.scalar.activation(TD[64 : 64 + H, :],
                                         Dst[g][64 : 64 + H, :], Tanh,
                                         scale=0.5)
                    nc.vector.scalar_tensor_tensor(
                        RT[g][64 : 64 + H, ncols], TB[64 : 64 + H, :], 1.0,
                        TD[64 : 64 + H, :], add, mult)

            # final FC + sigmoid; H_last lives in slot S%RB (= 0)
            fsl = S % RB
            for g in range(G):
                fcols = slice(fsl * GN, (fsl + 1) * GN)
                po = ppool.tile([128, GN], fp32, tag=f"zA{g}", name="po")
                ta_old = TAhist[g][-2]
                nc.tensor.matmul(po[0:1, 0:1], ta_old[0:1, 0:1],
                                 ta_old[0:1, 0:1], skip_group_check=True)
                nc.tensor.matmul(po[0:1, :], w_fc_sb[:], RT[g][:, fcols],
                                 skip_group_check=True)
                to_sb = wpool.tile([1, GN], fp32, tag=f"to{g}")
                # sigmoid(u) = 0.5 + 0.5*tanh(0.5*u); b_fc folded into w_fc
                nc.scalar.activation(to_sb[:], po[0:1, :], Tanh, scale=0.5)
                o_sb = wpool.tile([1, GN], fp32, tag=f"o{g}")
                nc.vector.tensor_scalar(o_sb[:], to_sb[:], 0.5, 0.5, mult, add)
                nc.gpsimd.dma_start(out_d[0:1, g * GN : (g + 1) * GN], o_sb[:])

    return nc


def _prep_inputs(x, W_ih, W_hh, b_ih, b_hh, W_fc, b_fc):
    """Host-side weight preprocessing + per-core sharding."""
    x = np.asarray(x, np.float32)
    W_ih = np.asarray(W_ih, np.float32)
    W_hh = np.asarray(W_hh, np.float32)
    b = np.asarray(b_ih, np.float32) + np.asarray(b_hh, np.float32)
    W_fc = np.asarray(W_fc, np.float32)

    # gate rows: i(0:50) f(50:100) g(100:150) o(150:200)
    row_scale = np.full(4 * H, 0.5, np.float32)
    row_scale[2 * H : 3 * H] = 1.0  # g rows use tanh directly
    W_hh_eff = (row_scale[:, None] * W_hh * 0.5).astype(np.float32)  # H=2h comp
    W_ih_eff = (row_scale * W_ih[:, 0]).astype(np.float32)
    b_eff = (row_scale * b).astype(np.float32)

    # stationary weights [KK, 128]: row 0 = x weights, row 1 = bias,
    # rows 64:114 = W_hh^T ; gate pair at cols 0:50 and 64:114
    def bank(g1, g2):
        w = np.zeros((KK, 128), np.float32)
        for col, lo in ((0, g1), (64, g2)):
            w[0, col : col + H] = b_eff[lo : lo + H]
            w[64 : 64 + H, col : col + H] = W_hh_eff[lo : lo + H].T
        return w

    w_ifb = bank(0, H)          # i cols 0:50, f cols 64:114
    w_gob = bank(2 * H, 3 * H)  # g cols 0:50, o cols 64:114

    def xvec(g1, g2):
        w = np.zeros((1, 128), np.float32)
        w[0, 0:H] = W_ih_eff[g1 : g1 + H]
        w[0, 64 : 64 + H] = W_ih_eff[g2 : g2 + H]
        return w
    w_fc_t = np.zeros((KK, 1), np.float32)
    w_fc_t[0, 0] = float(np.asarray(b_fc, np.float32).reshape(-1)[0])
    w_fc_t[64 : 64 + H, 0] = 0.5 * W_fc[0, :]
    b_fc_val = 0.0

    in_maps = []
    for c in range(NCORES):
        xs = x[c * BS : (c + 1) * BS, :, 0]          # [BS, S]
        in_maps.append({
            "xT": np.ascontiguousarray(xs.T),         # [S, BS]
            "w_ifb": w_ifb,
            "w_gob": w_gob,
            "w_fc": w_fc_t,
            "wx_if": xvec(0, H),
            "wx_go": xvec(2 * H, 3 * H),
        })
    return in_maps, b_fc_val


def _run(inputs, trace=False):
    from concourse.bass_utils import run_bass_kernel_spmd

    in_maps, b_fc_val = _prep_inputs(**inputs)
    key = "nc"
    if key not in _cache:
        _cache[key] = _build(b_fc_val)
    nc = _cache[key]
    res = run_bass_kernel_spmd(nc, in_maps, core_ids=list(range(NCORES)),
                               trace=trace)
    outs = [r["out"].reshape(BS) for r in res.results]
    full = np.concatenate(outs).reshape(B, 1).astype(np.float32)
    return full, res


def kernel(**inputs) -> np.ndarray:
    out, _ = _run(inputs, trace=False)
    return out



# revision 4
# speedup vs baseline: 12.2017x; 12.2017x over previous
"""LSTM (B=4096, S=512, I=1, H=50) Bass kernel for 8 TRN2 NeuronCores.

Strategy: data-parallel over batch (512 rows per core). Per core the scan
runs with hidden on SBUF partitions and batch on the free dim, so h comes
out of the elementwise stage already transposed for the next matmul.

Math tricks (all host-side weight preprocessing):
  - sigmoid(x) = (1 + tanh(x/2)) / 2  -> every gate is a single Tanh; all
    four gates of one step live in 2 ACT instructions.
  - State D = 2c and H = 2h absorb the /2 factors:
        D' = 0.5*(1+tf)*D + (1+ti)*tg        (3 scalar_tensor_tensor ops)
        H' = (1+to) * tanh(0.5*D')           (1 ACT + 1 STT op)
    with W_hh pre-scaled by 0.5 column-wise (H=2h input) and gate rows
    scaled 0.5 (i,f,o) / 1.0 (g).
  - x-projection and bias folded into the recurrence matmul by augmenting
    the state tile with an x-row and a ones-row.

Batch is split in two groups of 256 per core so the two dependency chains
pipeline across engines.

Host path (dominates wall time over the axon tunnel: ~70 ms RTT,
~130 MB/s):
  - the jitted shard_map executable is built once and cached in module
    state; warm calls only pay input transfer + execute + output fetch.
  - x ships as fp16 (4 MB instead of 8) and stays fp16 through the
    rank-1 x-projection matmul; weights are packed into two small
    replicated tensors.
"""

import numpy as np

B, S, H = 4096, 512, 50
NCORES = 8
BS = B // NCORES          # 512 batch rows per core
G = 2                     # pipeline groups per core
GN = BS // G              # 256 batch columns per group
KK = 114                  # rows: 0=ones/bias, 64:114 = H-state
RB = 32                   # ring slots / x-staging block

_cache = {}


def _build():
    import concourse.bass as bass
    import concourse.mybir as mybir
    from concourse.tile import TileContext
    from concourse.vector_clock import ScopedClock

    class TC1W(TileContext):
        # this walrus accepts only ONE sem wait per instruction; split any
        # instruction's extra waits onto preceding same-engine NOPs
        def _split_multiwaits(self):
            nc_ = self.nc
            cnt = 0
            for f_ in nc_.m.functions:
                for bb in f_.blocks:
                    il = list(bb.instructions)
                    out, changed = [], False
                    for ins in il:
                        si = ins.sync_info
                        if si is not None and si.on_wait and len(si.on_wait) > 1:
                            waits = list(si.on_wait)
                            for w in waits[:-1]:
                                cnt += 1
                                nop = mybir.InstNoOp(
                                    name=f"wsplit{cnt}", ins=[], outs=[])
                                nop.engine = ins.engine
                                nop.sync_info = mybir.SyncInfo(
                                    on_wait=[w], on_update=[])
                                out.append(nop)
                            si.on_wait = waits[-1:]
                            changed = True
                        out.append(ins)
                    if changed:
                        bb.instructions = out

        def _drain_and_barrier(self, tick_clock, wait_clock):
            nc_ = self.nc
            self._split_multiwaits()
            drain_inst = nc_.sync.drain()
            wait_clock.add_sem_waits(
                drain_inst.ins, ScopedClock({None: tick_clock.global_clock}))
            si = drain_inst.ins.sync_info
            waits = list(si.on_wait) if si is not None and si.on_wait else []
            if len(waits) > 1:
                si.on_wait = waits[:1]
                for w in waits[1:]:
                    d2 = nc_.sync.drain()
                    si2 = d2.ins.sync_info
                    if si2 is None:
                        d2.ins.sync_info = mybir.SyncInfo(on_wait=[w],
                                                          on_update=[])
                    else:
                        si2.on_wait = [w]
            nc_.all_engine_barrier()
            popped = nc_._tile_sem_poison_stack.pop()
            assert popped is self._sem_poison
            nc_.clear_and_free_semaphores(list(self.sems.allocated().values()))
            nc_.all_engine_barrier()

    fp32 = mybir.dt.float32
    fp16 = mybir.dt.float16
    Tanh = mybir.ActivationFunctionType.Tanh
    add = mybir.AluOpType.add
    mult = mybir.AluOpType.mult

    nc = bass.Bass("TRN2")

    xT = nc.dram_tensor("xT", [S, BS], fp16, kind="ExternalInput")
    # packed fp32 weights: rows 0:114 w_ifb, 114:228 w_gob, 228:342 w_fc
    wpack = nc.dram_tensor("wpack", [342, 128], fp32, kind="ExternalInput")
    # packed fp16 x-projection vectors: row 0 wx_if, row 1 wx_go
    wxpack = nc.dram_tensor("wxpack", [2, 128], fp16, kind="ExternalInput")
    out_d = nc.dram_tensor("out", [1, BS], fp32, kind="ExternalOutput")

    xT3 = xT.rearrange("(o s) b -> o s b", o=1)

    with TC1W(nc) as tc:
        with (
            tc.tile_pool(name="const", bufs=1) as cpool,
            tc.tile_pool(name="work", bufs=2) as wpool,
            tc.tile_pool(name="psum", bufs=2, space="PSUM") as ppool,
        ):
            w_ifb_sb = cpool.tile([KK, 128], fp32, tag="w_ifb")
            w_gob_sb = cpool.tile([KK, 128], fp32, tag="w_gob")
            w_fc_sb = cpool.tile([KK, 1], fp32, tag="w_fc")
            wx_if_sb = cpool.tile([1, 128], fp16, tag="wx_if")
            wx_go_sb = cpool.tile([1, 128], fp16, tag="wx_go")
            nc.gpsimd.dma_start(wx_if_sb[:], wxpack[0:1, :])
            nc.gpsimd.dma_start(wx_go_sb[:], wxpack[1:2, :])
            nc.gpsimd.dma_start(w_ifb_sb[:], wpack[0:KK, :])
            nc.gpsimd.dma_start(w_gob_sb[:], wpack[KK : 2 * KK, :])
            nc.gpsimd.dma_start(w_fc_sb[:], wpack[2 * KK : 3 * KK, 0:1])

            # ring tiles: row 0 = ones, rows 64:114 = H-state(=2h)
            # RB slots of GN columns each
            RT = [cpool.tile([KK, RB * GN], fp32, tag=f"RT{g}", name=f"RT{g}")
                  for g in range(G)]
            Dst = [cpool.tile([128, GN], fp32, tag=f"D{g}", name=f"D{g}")
                   for g in range(G)]
            XR = [cpool.tile([1, RB * GN], fp16, tag=f"XR{g}", name=f"XR{g}")
                  for g in range(G)]
            jnk = [cpool.tile([1, 1], fp32, tag=f"jnk{g}", name=f"jnk{g}")
                   for g in range(G)]
            for g in range(G):
                nc.vector.memset(RT[g][:], 0.0)
                nc.vector.memset(RT[g][0:1, :], 1.0)
                nc.vector.memset(Dst[g][:], 0.0)
                # x block 0: slots 0..RB-1
                nc.gpsimd.dma_start(
                    XR[g][0:1, :].rearrange("o (a b) -> o a b", b=GN),
                    xT3[0:1, 0:RB, g * GN : (g + 1) * GN])

            # wait-carrier dummies: absorb one DMA sem each on the PE
            pcar = ppool.tile([128, GN], fp32, tag="zA0", name="pcar")
            for src in (w_ifb_sb, w_gob_sb, w_fc_sb):
                nc.tensor.matmul(pcar[0:1, 0:1], src[0:1, 0:1],
                                 src[0:1, 0:1], skip_group_check=True)
            for src in (wx_if_sb, wx_go_sb, XR[0], XR[1]):
                nc.tensor.matmul(pcar[0:1, 0:1], src[0:1, 0:1],
                                 src[0:1, 0:1], skip_group_check=True)

            TAhist = {0: [], 1: []}
            for t in range(S):
                sl = t % RB
                sn = (t + 1) % RB
                for g in range(G):
                    cols = slice(sl * GN, (sl + 1) * GN)
                    ncols = slice(sn * GN, (sn + 1) * GN)
                    # stage next x block (one DMA per RB steps)
                    if t % RB == 0 and t + RB < S:
                        nc.gpsimd.dma_start(
                            XR[g][0:1, :].rearrange("o (a b) -> o a b", b=GN),
                            xT3[0:1, t + RB : t + 2 * RB,
                                g * GN : (g + 1) * GN])

                    zA = ppool.tile([128, GN], fp32, tag=f"zA{g}")
                    zB = ppool.tile([128, GN], fp32, tag=f"zB{g}")
                    if len(TAhist[g]) >= 2:
                        # PE carrier: absorb the ACT tick (zA/zB slot WAR)
                        ta_old = TAhist[g][-2]
                        nc.tensor.matmul(zA[0:1, 0:1], ta_old[0:1, 0:1],
                                         ta_old[0:1, 0:1],
                                         skip_group_check=True)
                    nc.tensor.matmul(zA[:], w_ifb_sb[:], RT[g][:, cols],
                                     start=True, stop=False)
                    nc.tensor.matmul(zA[:], wx_if_sb[:], XR[g][0:1, cols],
                                     start=False, stop=True)
                    nc.tensor.matmul(zB[:], w_gob_sb[:], RT[g][:, cols],
                                     start=True, stop=False)
                    nc.tensor.matmul(zB[:], wx_go_sb[:], XR[g][0:1, cols],
                                     start=False, stop=True)

                    # all-tanh gates: TA = [ti @0 ; tf @64], TB = [tg @0 ; to @64]
                    TA = wpool.tile([128, GN], fp32, tag=f"TA{g}")
                    TB = wpool.tile([128, GN], fp32, tag=f"TB{g}")
                    nc.scalar.activation(TA[:], zA[:], Tanh)
                    nc.scalar.activation(TB[:], zB[:], Tanh)
                    TAhist[g].append(TA)
                    # DVE carrier: absorb the PE tick (covers ring WAR for H2)
                    nc.vector.tensor_copy(jnk[g][0:1, 0:1], zB[0:1, 0:1])

                    # D' = 0.5*(1+tf)*D + (1+ti)*tg      (state D = 2c @64)
                    Bt = wpool.tile([H, GN], fp32, tag=f"Bt{g}")
                    At = wpool.tile([H, GN], fp32, tag=f"At{g}")
                    nc.vector.scalar_tensor_tensor(
                        Bt[:], TA[64 : 64 + H, :], 1.0,
                        Dst[g][64 : 64 + H, :], add, mult)
                    nc.vector.scalar_tensor_tensor(
                        At[:], TA[0:H, :], 1.0, TB[0:H, :], add, mult)
                    nc.vector.scalar_tensor_tensor(
                        Dst[g][64 : 64 + H, :], Bt[:], 0.5, At[:], mult, add)

                    # H' = (1+to) * tanh(0.5*D') -> ring slot t+1, rows 64:114
                    TD = wpool.tile([128, GN], fp32, tag=f"TD{g}")
                    nc.scalar.activation(TD[64 : 64 + H, :],
                                         Dst[g][64 : 64 + H, :], Tanh,
                                         scale=0.5)
                    nc.vector.scalar_tensor_tensor(
                        RT[g][64 : 64 + H, ncols], TB[64 : 64 + H, :], 1.0,
                        TD[64 : 64 + H, :], add, mult)

            # final FC + sigmoid; H_last lives in slot S%RB (= 0)
            fsl = S % RB
            for g in range(G):
                fcols = slice(fsl * GN, (fsl + 1) * GN)
                po = ppool.tile([128, GN], fp32, tag=f"zA{g}", name="po")
                ta_old = TAhist[g][-2]
                nc.tensor.matmul(po[0:1, 0:1], ta_old[0:1, 0:1],
                                 ta_old[0:1, 0:1], skip_group_check=True)
                nc.tensor.matmul(po[0:1, :], w_fc_sb[:], RT[g][:, fcols],
                                 skip_group_check=True)
                to_sb = wpool.tile([1, GN], fp32, tag=f"to{g}")
                # sigmoid(u) = 0.5 + 0.5*tanh(0.5*u); b_fc folded into w_fc
                nc.scalar.activation(to_sb[:], po[0:1, :], Tanh, scale=0.5)
                o_sb = wpool.tile([1, GN], fp32, tag=f"o{g}")
                nc.vector.tensor_scalar(o_sb[:], to_sb[:], 0.5, 0.5, mult, add)
                nc.gpsimd.dma_start(out_d[0:1, g * GN : (g + 1) * GN], o_sb[:])

    return nc


def _prep_global(x, W_ih, W_hh, b_ih, b_hh, W_fc, b_fc):
    """Host-side weight preprocessing; returns {name: array}.

    xT is the global sharded array (axis 0 = 8 core shards); wpack/wxpack
    are single replicated arrays.
    """
    x = np.asarray(x, np.float32)
    W_ih = np.asarray(W_ih, np.float32)
    W_hh = np.asarray(W_hh, np.float32)
    b = np.asarray(b_ih, np.float32) + np.asarray(b_hh, np.float32)
    W_fc = np.asarray(W_fc, np.float32)

    # gate rows: i(0:50) f(50:100) g(100:150) o(150:200)
    row_scale = np.full(4 * H, 0.5, np.float32)
    row_scale[2 * H : 3 * H] = 1.0  # g rows use tanh directly
    W_hh_eff = (row_scale[:, None] * W_hh * 0.5).astype(np.float32)  # H=2h comp
    W_ih_eff = (row_scale * W_ih[:, 0]).astype(np.float32)
    b_eff = (row_scale * b).astype(np.float32)

    # stationary weights [KK, 128]: row 0 = bias (vs ones),
    # rows 64:114 = W_hh^T ; gate pair at cols 0:50 and 64:114
    def bank(g1, g2):
        w = np.zeros((KK, 128), np.float32)
        for col, lo in ((0, g1), (64, g2)):
            w[0, col : col + H] = b_eff[lo : lo + H]
            w[64 : 64 + H, col : col + H] = W_hh_eff[lo : lo + H].T
        return w

    wpack = np.zeros((342, 128), np.float32)
    wpack[0:KK] = bank(0, H)                   # i cols 0:50, f cols 64:114
    wpack[KK : 2 * KK] = bank(2 * H, 3 * H)    # g cols 0:50, o cols 64:114
    # w_fc block: row 0 = b_fc (vs ones), rows 64:114 = 0.5*W_fc
    wpack[2 * KK, 0] = float(np.asarray(b_fc, np.float32).reshape(-1)[0])
    wpack[2 * KK + 64 : 2 * KK + 64 + H, 0] = 0.5 * W_fc[0, :]

    wxpack = np.zeros((2, 128), np.float16)
    for r, lo in ((0, (0, H)), (1, (2 * H, 3 * H))):
        g1, g2 = lo[0], lo[1]
        wxpack[r, 0:H] = W_ih_eff[g1 : g1 + H]
        wxpack[r, 64 : 64 + H] = W_ih_eff[g2 : g2 + H]

    # xT global: rows [c*S:(c+1)*S] = x[c*BS:(c+1)*BS, :, 0].T  (fp16)
    xT_g = np.ascontiguousarray(
        x.reshape(NCORES, BS, S).transpose(0, 2, 1).astype(np.float16)
    ).reshape(NCORES * S, BS)

    return {"xT": xT_g, "wpack": wpack, "wxpack": wxpack}


def _get_runner():
    """Build (once) and cache the jitted shard_map executable."""
    if "runner" in _cache:
        return _cache["runner"]

    import jax
    from jax.sharding import Mesh, PartitionSpec
    from jax.experimental.shard_map import shard_map
    from concourse import bass2jax
    import concourse.mybir as mybir

    nc = _build()
    bass2jax.install_neuronx_cc_hook()
    partition_name = (nc.partition_id_tensor.name
                      if nc.partition_id_tensor else None)

    in_names, out_names, out_avals, zero_outs = [], [], [], []
    for alloc in nc.m.functions[0].allocations:
        if not isinstance(alloc, mybir.MemoryLocationSet):
            continue
        name = alloc.memorylocations[0].name
        if alloc.kind == "ExternalInput":
            if name != partition_name:
                in_names.append(name)
        elif alloc.kind == "ExternalOutput":
            out_names.append(name)
            shape = tuple(alloc.tensor_shape)
            dtype = mybir.dt.np(alloc.dtype)
            out_avals.append(jax.core.ShapedArray(shape, dtype))
            zero_outs.append(np.zeros(shape, dtype))
    n_params = len(in_names)
    n_outs = len(out_avals)
    in_names_all = list(in_names) + out_names
    if partition_name is not None:
        in_names_all.append(partition_name)
    donate = tuple(range(n_params, n_params + n_outs))

    def _body(*args):
        operands = list(args)
        if partition_name is not None:
            operands.append(bass2jax.partition_id_tensor())
        outs = bass2jax._bass_exec_p.bind(
            *operands,
            out_avals=tuple(out_avals),
            in_names=tuple(in_names_all),
            out_names=tuple(out_names),
            lowering_input_output_aliases=(),
            sim_require_finite=True,
            sim_require_nnan=True,
            nc=nc,
        )
        return tuple(outs)

    devices = jax.devices()[:NCORES]
    assert len(devices) == NCORES, (
        f"need {NCORES} devices, got {len(jax.devices())}")
    mesh = Mesh(np.asarray(devices), ("core",))
    # xT sharded by core; small weight packs replicated
    spec_by_name = {"xT": PartitionSpec("core")}
    in_specs = tuple(
        [spec_by_name.get(n, PartitionSpec()) for n in in_names]
        + [PartitionSpec("core")] * n_outs)
    out_specs = (PartitionSpec("core"),) * len(out_names)
    sharded = jax.jit(
        shard_map(_body, mesh=mesh, in_specs=in_specs, out_specs=out_specs,
                  check_rep=False),
        donate_argnums=donate, keep_unused=True,
    )

    def run(global_in: dict):
        args = [global_in[name] for name in in_names]
        zeros = [np.zeros((NCORES * z.shape[0], *z.shape[1:]), z.dtype)
                 for z in zero_outs]
        out_arrs = sharded(*args, *zeros)
        # single output "out": global [NCORES, BS]
        return np.asarray(out_arrs[0])

    _cache["runner"] = run
    return run


def kernel(**inputs) -> np.ndarray:
    run = _get_runner()
    global_in = _prep_global(**inputs)
    out = run(global_in)
    return out.reshape(B, 1).astype(np.float32)


# revision 11
# speedup vs baseline: 15.0782x; 1.2357x over previous
"""LSTM (B=4096, S=512, I=1, H=50) Bass kernel for 8 TRN2 NeuronCores.

Strategy: data-parallel over batch (512 rows per core). Per core the scan
runs with hidden on SBUF partitions and batch on the free dim, so h comes
out of the elementwise stage already transposed for the next matmul.

Math tricks (all host-side weight preprocessing):
  - sigmoid(x) = (1 + tanh(x/2)) / 2  -> every gate is a single Tanh; all
    four gates of one step live in 2 ACT instructions.
  - State D = 2c and H = 2h absorb the /2 factors:
        D' = 0.5*(1+tf)*D + (1+ti)*tg        (3 scalar_tensor_tensor ops)
        H' = (1+to) * tanh(0.5*D')           (1 ACT + 1 STT op)
    with W_hh pre-scaled by 0.5 column-wise (H=2h input) and gate rows
    scaled 0.5 (i,f,o) / 1.0 (g).
  - x-projection and bias folded into the recurrence matmul: the ring tile
    carries a ones-row (bias) and an x-row, so each step is exactly two
    128x256 matmuls per group.

Batch is split in two groups of 256 per core so the two dependency chains
pipeline across engines.

x path: ships as fp16 in natural [batch, steps] layout (no host
transpose); on device a DMA-xbar transpose stages [128 steps, 512 batch]
blocks, a DVE copy upcasts to fp32, and SBUF->SBUF DMAs scatter 32-step
windows into the ring's x-row, all double-buffered ahead of the compute.

Host path (dominates wall time over the axon tunnel: ~70 ms RTT,
~130 MB/s): the jitted shard_map executable is built once and cached in
module state; warm calls only pay input transfer + execute + output
fetch.
"""

import numpy as np

B, S, H = 4096, 512, 50
NCORES = 8
BS = B // NCORES          # 512 batch rows per core
G = 2                     # pipeline groups per core
GN = BS // G              # 256 batch columns per group
KK = 114                  # rows: 0=ones/bias, 1=x, 64:114 = H-state
RB = 64                   # ring slots (2 x 32-step scatter windows)
XBLK = 128                # steps per xbar-transpose block
XW = 32                   # steps per ring scatter window

_cache = {}


def _build():
    import concourse.bass as bass
    import concourse.mybir as mybir
    from concourse.tile import TileContext
    from concourse.vector_clock import ScopedClock

    class TC1W(TileContext):
        # this walrus accepts only ONE sem wait per instruction; split any
        # instruction's extra waits onto preceding same-engine NOPs
        def _split_multiwaits(self):
            nc_ = self.nc
            cnt = 0
            for f_ in nc_.m.functions:
                for bb in f_.blocks:
                    il = list(bb.instructions)
                    out, changed = [], False
                    for ins in il:
                        si = ins.sync_info
                        if si is not None and si.on_wait and len(si.on_wait) > 1:
                            waits = list(si.on_wait)
                            for w in waits[:-1]:
                                cnt += 1
                                nop = mybir.InstNoOp(
                                    name=f"wsplit{cnt}", ins=[], outs=[])
                                nop.engine = ins.engine
                                nop.sync_info = mybir.SyncInfo(
                                    on_wait=[w], on_update=[])
                                out.append(nop)
                            si.on_wait = waits[-1:]
                            changed = True
                        out.append(ins)
                    if changed:
                        bb.instructions = out

        def _drain_and_barrier(self, tick_clock, wait_clock):
            nc_ = self.nc
            self._split_multiwaits()
            drain_inst = nc_.sync.drain()
            wait_clock.add_sem_waits(
                drain_inst.ins, ScopedClock({None: tick_clock.global_clock}))
            si = drain_inst.ins.sync_info
            waits = list(si.on_wait) if si is not None and si.on_wait else []
            if len(waits) > 1:
                si.on_wait = waits[:1]
                for w in waits[1:]:
                    d2 = nc_.sync.drain()
                    si2 = d2.ins.sync_info
                    if si2 is None:
                        d2.ins.sync_info = mybir.SyncInfo(on_wait=[w],
                                                          on_update=[])
                    else:
                        si2.on_wait = [w]
            nc_.all_engine_barrier()
            popped = nc_._tile_sem_poison_stack.pop()
            assert popped is self._sem_poison
            nc_.clear_and_free_semaphores(list(self.sems.allocated().values()))
            nc_.all_engine_barrier()

    fp32 = mybir.dt.float32
    fp16 = mybir.dt.float16
    Tanh = mybir.ActivationFunctionType.Tanh
    add = mybir.AluOpType.add
    mult = mybir.AluOpType.mult

    nc = bass.Bass("TRN2")

    # x in natural layout [batch, steps]; transposed on-device via DMA xbar
    xn = nc.dram_tensor("xn", [BS, S], fp16, kind="ExternalInput")
    # packed fp32 weights: rows 0:114 w_ifb, 114:228 w_gob, 228:342 w_fc
    wpack = nc.dram_tensor("wpack", [342, 128], fp32, kind="ExternalInput")
    out_d = nc.dram_tensor("out", [1, BS], fp32, kind="ExternalOutput")

    with TC1W(nc) as tc:
        with (
            tc.tile_pool(name="const", bufs=1) as cpool,
            tc.tile_pool(name="work", bufs=2) as wpool,
            tc.tile_pool(name="psum", bufs=2, space="PSUM") as ppool,
        ):
            w_ifb_sb = cpool.tile([KK, 128], fp32, tag="w_ifb")
            w_gob_sb = cpool.tile([KK, 128], fp32, tag="w_gob")
            w_fc_sb = cpool.tile([KK, 1], fp32, tag="w_fc")
            nc.gpsimd.dma_start(w_ifb_sb[:], wpack[0:KK, :])
            nc.gpsimd.dma_start(w_gob_sb[:], wpack[KK : 2 * KK, :])
            nc.gpsimd.dma_start(w_fc_sb[:], wpack[2 * KK : 3 * KK, 0:1])

            # ring tiles: row 0 = ones, row 1 = x_t, rows 64:114 = H(=2h)
            RT = [cpool.tile([KK, RB * GN], fp32, tag=f"RT{g}", name=f"RT{g}")
                  for g in range(G)]
            Dst = [cpool.tile([128, GN], fp32, tag=f"D{g}", name=f"D{g}")
                   for g in range(G)]
            # x staging: xbar-transposed fp16 blocks + fp32 upcast blocks
            XB = [cpool.tile([XBLK, BS], fp16, tag=f"XB{k}", name=f"XB{k}")
                  for k in range(2)]
            XC = [cpool.tile([XBLK, BS], fp32, tag=f"XC{k}", name=f"XC{k}")
                  for k in range(2)]
            jnk = [cpool.tile([1, 1], fp32, tag=f"jnk{g}", name=f"jnk{g}")
                   for g in range(G)]
            for g in range(G):
                nc.vector.memset(RT[g][:], 0.0)
                nc.vector.memset(RT[g][0:1, :], 1.0)
                nc.vector.memset(Dst[g][:], 0.0)

            def scatter_x(t0):
                # x rows for steps [t0, t0+XW) -> ring x-row slots
                blk = t0 // XBLK
                r0 = t0 % XBLK
                s0 = t0 % RB
                for g in range(G):
                    nc.sync.dma_start(
                        RT[g][1:2, s0 * GN : (s0 + XW) * GN].rearrange(
                            "o (a b) -> o a b", b=GN),
                        XC[blk % 2][r0 : r0 + XW, g * GN : (g + 1) * GN])

            # prefill: block 0 transpose + upcast + two scatter windows
            nc.sync.dma_start(XB[0][:], xn[:, 0:XBLK], transpose=True)
            nc.vector.tensor_copy(XC[0][:], XB[0][:])
            scatter_x(0)
            scatter_x(XW)

            # wait-carrier dummies: absorb one DMA sem each on the PE
            pcar = ppool.tile([128, GN], fp32, tag="zA0", name="pcar")
            for src in (w_ifb_sb, w_gob_sb, w_fc_sb):
                nc.tensor.matmul(pcar[0:1, 0:1], src[0:1, 0:1],
                                 src[0:1, 0:1], skip_group_check=True)

            TAhist = {0: [], 1: []}
            for t in range(S):
                sl = t % RB
                sn = (t + 1) % RB
                # x staging pipeline (all hidden behind ~32 steps of compute)
                if t % XBLK == 0 and t + XBLK < S:
                    k = (t // XBLK + 1) % 2
                    nc.sync.dma_start(XB[k][:],
                                      xn[:, t + XBLK : t + 2 * XBLK],
                                      transpose=True)
                if t % XBLK == 64 and t + 64 < S:
                    k = (t // XBLK + 1) % 2
                    nc.vector.tensor_copy(XC[k][:], XB[k][:])
                if t % XW == 0 and t >= XW and t + XW < S:
                    scatter_x(t + XW)

                for g in range(G):
                    cols = slice(sl * GN, (sl + 1) * GN)
                    ncols = slice(sn * GN, (sn + 1) * GN)

                    zA = ppool.tile([128, GN], fp32, tag=f"zA{g}")
                    zB = ppool.tile([128, GN], fp32, tag=f"zB{g}")
                    if len(TAhist[g]) >= 2:
                        # PE carrier: absorb the ACT tick (zA/zB slot WAR)
                        ta_old = TAhist[g][-2]
                        nc.tensor.matmul(zA[0:1, 0:1], ta_old[0:1, 0:1],
                                         ta_old[0:1, 0:1],
                                         skip_group_check=True)
                    nc.tensor.matmul(zA[:], w_ifb_sb[:], RT[g][:, cols],
                                     skip_group_check=True)
                    nc.tensor.matmul(zB[:], w_gob_sb[:], RT[g][:, cols],
                                     skip_group_check=True)

                    # all-tanh gates: TA = [ti @0 ; tf @64], TB = [tg @0 ; to @64]
                    TA = wpool.tile([128, GN], fp32, tag=f"TA{g}")
                    TB = wpool.tile([128, GN], fp32, tag=f"TB{g}")
                    nc.scalar.activation(TA[:], zA[:], Tanh)
                    nc.scalar.activation(TB[:], zB[:], Tanh)
                    TAhist[g].append(TA)
                    # DVE carrier: absorb the PE tick (covers ring WAR for H2)
                    nc.vector.tensor_copy(jnk[g][0:1, 0:1], zB[0:1, 0:1])

                    # D' = 0.5*(1+tf)*D + (1+ti)*tg      (state D = 2c @64)
                    Bt = wpool.tile([H, GN], fp32, tag=f"Bt{g}")
                    At = wpool.tile([H, GN], fp32, tag=f"At{g}")
                    nc.vector.scalar_tensor_tensor(
                        Bt[:], TA[64 : 64 + H, :], 1.0,
                        Dst[g][64 : 64 + H, :], add, mult)
                    nc.vector.scalar_tensor_tensor(
                        At[:], TA[0:H, :], 1.0, TB[0:H, :], add, mult)
                    nc.vector.scalar_tensor_tensor(
                        Dst[g][64 : 64 + H, :], Bt[:], 0.5, At[:], mult, add)

                    # H' = (1+to) * tanh(0.5*D') -> ring slot t+1, rows 64:114
                    TD = wpool.tile([128, GN], fp32, tag=f"TD{g}")
                    nc.scalar.activation(TD[64 : 64 + H, :],
                                         Dst[g][64 : 64 + H, :], Tanh,
                                         scale=0.5)
                    nc.vector.scalar_tensor_tensor(
                        RT[g][64 : 64 + H, ncols], TB[64 : 64 + H, :], 1.0,
                        TD[64 : 64 + H, :], add, mult)

            # final FC + sigmoid; H_last lives in slot S%RB (= 0)
            fsl = S % RB
            for g in range(G):
                fcols = slice(fsl * GN, (fsl + 1) * GN)
                po = ppool.tile([128, GN], fp32, tag=f"zA{g}", name="po")
                ta_old = TAhist[g][-2]
                nc.tensor.matmul(po[0:1, 0:1], ta_old[0:1, 0:1],
                                 ta_old[0:1, 0:1], skip_group_check=True)
                nc.tensor.matmul(po[0:1, :], w_fc_sb[:], RT[g][:, fcols],
                                 skip_group_check=True)
                to_sb = wpool.tile([1, GN], fp32, tag=f"to{g}")
                # sigmoid(u) = 0.5 + 0.5*tanh(0.5*u); b_fc folded into w_fc
                nc.scalar.activation(to_sb[:], po[0:1, :], Tanh, scale=0.5)
                o_sb = wpool.tile([1, GN], fp32, tag=f"o{g}")
                nc.vector.tensor_scalar(o_sb[:], to_sb[:], 0.5, 0.5, mult, add)
                nc.gpsimd.dma_start(out_d[0:1, g * GN : (g + 1) * GN], o_sb[:])

    return nc


def _prep_global(x, W_ih, W_hh, b_ih, b_hh, W_fc, b_fc):
    """Host-side weight preprocessing; returns {name: array}.

    xn is the global sharded array (axis 0 = 8 core shards of [BS, S]);
    wpack is a single replicated array.
    """
    x = np.asarray(x, np.float32)
    W_ih = np.asarray(W_ih, np.float32)
    W_hh = np.asarray(W_hh, np.float32)
    b = np.asarray(b_ih, np.float32) + np.asarray(b_hh, np.float32)
    W_fc = np.asarray(W_fc, np.float32)

    # gate rows: i(0:50) f(50:100) g(100:150) o(150:200)
    row_scale = np.full(4 * H, 0.5, np.float32)
    row_scale[2 * H : 3 * H] = 1.0  # g rows use tanh directly
    W_hh_eff = (row_scale[:, None] * W_hh * 0.5).astype(np.float32)  # H=2h comp
    W_ih_eff = (row_scale * W_ih[:, 0]).astype(np.float32)
    b_eff = (row_scale * b).astype(np.float32)

    # stationary weights [KK, 128]: row 0 = bias (vs ones), row 1 = x
    # weights, rows 64:114 = W_hh^T ; gate pair at cols 0:50 and 64:114
    def bank(g1, g2):
        w = np.zeros((KK, 128), np.float32)
        for col, lo in ((0, g1), (64, g2)):
            w[0, col : col + H] = b_eff[lo : lo + H]
            w[1, col : col + H] = W_ih_eff[lo : lo + H]
            w[64 : 64 + H, col : col + H] = W_hh_eff[lo : lo + H].T
        return w

    wpack = np.zeros((342, 128), np.float32)
    wpack[0:KK] = bank(0, H)                   # i cols 0:50, f cols 64:114
    wpack[KK : 2 * KK] = bank(2 * H, 3 * H)    # g cols 0:50, o cols 64:114
    # w_fc block: row 0 = b_fc (vs ones), rows 64:114 = 0.5*W_fc
    wpack[2 * KK, 0] = float(np.asarray(b_fc, np.float32).reshape(-1)[0])
    wpack[2 * KK + 64 : 2 * KK + 64 + H, 0] = 0.5 * W_fc[0, :]

    # x natural layout [B, S] fp16; axis 0 shards into 8 x [BS, S]
    xn_g = x.reshape(B, S).astype(np.float16)

    return {"xn": xn_g, "wpack": wpack}


def _get_runner():
    """Build (once) and cache the jitted shard_map executable."""
    if "runner" in _cache:
        return _cache["runner"]

    import jax
    from jax.sharding import Mesh, PartitionSpec
    from jax.experimental.shard_map import shard_map
    from concourse import bass2jax
    import concourse.mybir as mybir

    nc = _build()
    bass2jax.install_neuronx_cc_hook()
    partition_name = (nc.partition_id_tensor.name
                      if nc.partition_id_tensor else None)

    in_names, out_names, out_avals, zero_outs = [], [], [], []
    for alloc in nc.m.functions[0].allocations:
        if not isinstance(alloc, mybir.MemoryLocationSet):
            continue
        name = alloc.memorylocations[0].name
        if alloc.kind == "ExternalInput":
            if name != partition_name:
                in_names.append(name)
        elif alloc.kind == "ExternalOutput":
            out_names.append(name)
            shape = tuple(alloc.tensor_shape)
            dtype = mybir.dt.np(alloc.dtype)
            out_avals.append(jax.core.ShapedArray(shape, dtype))
            zero_outs.append(np.zeros(shape, dtype))
    n_params = len(in_names)
    n_outs = len(out_avals)
    in_names_all = list(in_names) + out_names
    if partition_name is not None:
        in_names_all.append(partition_name)
    donate = tuple(range(n_params, n_params + n_outs))

    def _body(*args):
        operands = list(args)
        if partition_name is not None:
            operands.append(bass2jax.partition_id_tensor())
        outs = bass2jax._bass_exec_p.bind(
            *operands,
            out_avals=tuple(out_avals),
            in_names=tuple(in_names_all),
            out_names=tuple(out_names),
            lowering_input_output_aliases=(),
            sim_require_finite=True,
            sim_require_nnan=True,
            nc=nc,
        )
        return tuple(outs)

    devices = jax.devices()[:NCORES]
    assert len(devices) == NCORES, (
        f"need {NCORES} devices, got {len(jax.devices())}")
    mesh = Mesh(np.asarray(devices), ("core",))
    # x sharded by core; small weight pack replicated
    spec_by_name = {"xn": PartitionSpec("core")}
    in_specs = tuple(
        [spec_by_name.get(n, PartitionSpec()) for n in in_names]
        + [PartitionSpec("core")] * n_outs)
    out_specs = (PartitionSpec("core"),) * len(out_names)
    sharded = jax.jit(
        shard_map(_body, mesh=mesh, in_specs=in_specs, out_specs=out_specs,
                  check_rep=False),
        donate_argnums=donate, keep_unused=True,
    )

    def run(global_in: dict):
        args = [global_in[name] for name in in_names]
        zeros = [np.zeros((NCORES * z.shape[0], *z.shape[1:]), z.dtype)
                 for z in zero_outs]
        out_arrs = sharded(*args, *zeros)
        # single output "out": global [NCORES, BS]
        return np.asarray(out_arrs[0])

    _cache["runner"] = run
    return run


def kernel(**inputs) -> np.ndarray:
    run = _get_runner()
    global_in = _prep_global(**inputs)
    out = run(global_in)
    return out.reshape(B, 1).astype(np.float32)


# revision 12
# speedup vs baseline: 15.1645x; 1.0057x over previous
"""LSTM (B=4096, S=512, I=1, H=50) Bass kernel for 8 TRN2 NeuronCores.

Strategy: data-parallel over batch (512 rows per core). Per core the scan
runs with hidden on SBUF partitions and batch on the free dim, so h comes
out of the elementwise stage already transposed for the next matmul.

Math tricks (all host-side weight preprocessing):
  - sigmoid(x) = (1 + tanh(x/2)) / 2  -> every gate is a single Tanh; all
    four gates of one step live in 2 ACT instructions.
  - State D = 2c and H = 2h absorb the /2 factors:
        D' = 0.5*(1+tf)*D + (1+ti)*tg        (3 scalar_tensor_tensor ops)
        H' = (1+to) * tanh(0.5*D')           (1 ACT + 1 STT op)
    with W_hh pre-scaled by 0.5 column-wise (H=2h input) and gate rows
    scaled 0.5 (i,f,o) / 1.0 (g).
  - x-projection and bias folded into the recurrence matmul: the ring tile
    carries a ones-row (bias) and an x-row, so each step is exactly two
    128x256 matmuls per group.

Batch is split in two groups of 256 per core so the two dependency chains
pipeline across engines.

x path: ships as fp16 in natural [batch, steps] layout (no host
transpose); on device a DMA-xbar transpose stages [128 steps, 512 batch]
blocks, a DVE copy upcasts to fp32, and SBUF->SBUF DMAs scatter 32-step
windows into the ring's x-row, all double-buffered ahead of the compute.

Host path (dominates wall time over the axon tunnel: ~70 ms RTT,
~130 MB/s): the jitted shard_map executable is built once and cached in
module state; warm calls only pay input transfer + execute + output
fetch.
"""

import numpy as np

B, S, H = 4096, 512, 50
NCORES = 8
BS = B // NCORES          # 512 batch rows per core
G = 2                     # pipeline groups per core
GN = BS // G              # 256 batch columns per group
KK = 114                  # rows: 0=ones/bias, 1=x, 64:114 = H-state
RB = 64                   # ring slots (2 x 32-step scatter windows)
XBLK = 128                # steps per xbar-transpose block
XW = 32                   # steps per ring scatter window

_cache = {}


def _build():
    import concourse.bass as bass
    import concourse.mybir as mybir
    from concourse.tile import TileContext
    from concourse.vector_clock import ScopedClock

    class TC1W(TileContext):
        # this walrus accepts only ONE sem wait per instruction; split any
        # instruction's extra waits onto preceding same-engine NOPs
        def _split_multiwaits(self):
            nc_ = self.nc
            cnt = 0
            for f_ in nc_.m.functions:
                for bb in f_.blocks:
                    il = list(bb.instructions)
                    out, changed = [], False
                    for ins in il:
                        si = ins.sync_info
                        if si is not None and si.on_wait and len(si.on_wait) > 1:
                            waits = list(si.on_wait)
                            for w in waits[:-1]:
                                cnt += 1
                                nop = mybir.InstNoOp(
                                    name=f"wsplit{cnt}", ins=[], outs=[])
                                nop.engine = ins.engine
                                nop.sync_info = mybir.SyncInfo(
                                    on_wait=[w], on_update=[])
                                out.append(nop)
                            si.on_wait = waits[-1:]
                            changed = True
                        out.append(ins)
                    if changed:
                        bb.instructions = out

        def _drain_and_barrier(self, tick_clock, wait_clock):
            nc_ = self.nc
            self._split_multiwaits()
            drain_inst = nc_.sync.drain()
            wait_clock.add_sem_waits(
                drain_inst.ins, ScopedClock({None: tick_clock.global_clock}))
            si = drain_inst.ins.sync_info
            waits = list(si.on_wait) if si is not None and si.on_wait else []
            if len(waits) > 1:
                si.on_wait = waits[:1]
                for w in waits[1:]:
                    d2 = nc_.sync.drain()
                    si2 = d2.ins.sync_info
                    if si2 is None:
                        d2.ins.sync_info = mybir.SyncInfo(on_wait=[w],
                                                          on_update=[])
                    else:
                        si2.on_wait = [w]
            nc_.all_engine_barrier()
            popped = nc_._tile_sem_poison_stack.pop()
            assert popped is self._sem_poison
            nc_.clear_and_free_semaphores(list(self.sems.allocated().values()))
            nc_.all_engine_barrier()

    fp32 = mybir.dt.float32
    fp16 = mybir.dt.float16
    Tanh = mybir.ActivationFunctionType.Tanh
    add = mybir.AluOpType.add
    mult = mybir.AluOpType.mult

    nc = bass.Bass("TRN2")

    # x in natural layout [batch, steps]; transposed on-device via DMA xbar
    xn = nc.dram_tensor("xn", [BS, S], fp16, kind="ExternalInput")
    # packed fp32 weights: rows 0:114 w_ifb, 114:228 w_gob, 228:342 w_fc
    wpack = nc.dram_tensor("wpack", [342, 128], fp32, kind="ExternalInput")
    out_d = nc.dram_tensor("out", [1, BS], fp32, kind="ExternalOutput")

    with TC1W(nc) as tc:
        with (
            tc.tile_pool(name="const", bufs=1) as cpool,
            tc.tile_pool(name="work", bufs=2) as wpool,
            tc.tile_pool(name="psum", bufs=2, space="PSUM") as ppool,
        ):
            w_ifb_sb = cpool.tile([KK, 128], fp32, tag="w_ifb")
            w_gob_sb = cpool.tile([KK, 128], fp32, tag="w_gob")
            w_fc_sb = cpool.tile([KK, 1], fp32, tag="w_fc")
            nc.gpsimd.dma_start(w_ifb_sb[:], wpack[0:KK, :])
            nc.gpsimd.dma_start(w_gob_sb[:], wpack[KK : 2 * KK, :])
            nc.gpsimd.dma_start(w_fc_sb[:], wpack[2 * KK : 3 * KK, 0:1])

            # ring tiles: row 0 = ones, row 1 = x_t, rows 64:114 = H(=2h)
            RT = [cpool.tile([KK, RB * GN], fp32, tag=f"RT{g}", name=f"RT{g}")
                  for g in range(G)]
            Dst = [cpool.tile([128, GN], fp32, tag=f"D{g}", name=f"D{g}")
                   for g in range(G)]
            # x staging: xbar-transposed fp16 blocks + fp32 upcast blocks
            XB = [cpool.tile([XBLK, BS], fp16, tag=f"XB{k}", name=f"XB{k}")
                  for k in range(2)]
            XC = [cpool.tile([XBLK, BS], fp32, tag=f"XC{k}", name=f"XC{k}")
                  for k in range(2)]
            jnk = [cpool.tile([1, 1], fp32, tag=f"jnk{g}", name=f"jnk{g}")
                   for g in range(G)]
            for g in range(G):
                nc.vector.memset(RT[g][:], 0.0)
                nc.vector.memset(RT[g][0:1, :], 1.0)
                nc.vector.memset(Dst[g][:], 0.0)

            def scatter_x(t0):
                # x rows for steps [t0, t0+XW) -> ring x-row slots
                blk = t0 // XBLK
                r0 = t0 % XBLK
                s0 = t0 % RB
                for g in range(G):
                    nc.sync.dma_start(
                        RT[g][1:2, s0 * GN : (s0 + XW) * GN].rearrange(
                            "o (a b) -> o a b", b=GN),
                        XC[blk % 2][r0 : r0 + XW, g * GN : (g + 1) * GN])

            # prefill: block 0 transpose + upcast + two scatter windows
            nc.sync.dma_start(XB[0][:], xn[:, 0:XBLK], transpose=True)
            nc.vector.tensor_copy(XC[0][:], XB[0][:])
            scatter_x(0)
            scatter_x(XW)

            # wait-carrier dummies: absorb one DMA sem each on the PE
            pcar = ppool.tile([128, GN], fp32, tag="zA0", name="pcar")
            for src in (w_ifb_sb, w_gob_sb, w_fc_sb):
                nc.tensor.matmul(pcar[0:1, 0:1], src[0:1, 0:1],
                                 src[0:1, 0:1], skip_group_check=True)

            TAhist = {0: [], 1: []}
            for t in range(S):
                sl = t % RB
                sn = (t + 1) % RB
                # x staging pipeline (all hidden behind ~32 steps of compute)
                if t % XBLK == 0 and t + XBLK < S:
                    k = (t // XBLK + 1) % 2
                    nc.sync.dma_start(XB[k][:],
                                      xn[:, t + XBLK : t + 2 * XBLK],
                                      transpose=True)
                if t % XBLK == 64 and t + 64 < S:
                    k = (t // XBLK + 1) % 2
                    nc.vector.tensor_copy(XC[k][:], XB[k][:])
                if t % XW == 0 and t >= XW and t + XW < S:
                    scatter_x(t + XW)

                for g in range(G):
                    cols = slice(sl * GN, (sl + 1) * GN)
                    ncols = slice(sn * GN, (sn + 1) * GN)

                    zA = ppool.tile([128, GN], fp32, tag=f"zA{g}")
                    zB = ppool.tile([128, GN], fp32, tag=f"zB{g}")
                    if len(TAhist[g]) >= 2:
                        # PE carrier: absorb the ACT tick (zA/zB slot WAR)
                        ta_old = TAhist[g][-2]
                        nc.tensor.matmul(zA[0:1, 0:1], ta_old[0:1, 0:1],
                                         ta_old[0:1, 0:1],
                                         skip_group_check=True)
                    nc.tensor.matmul(zA[:], w_ifb_sb[:], RT[g][:, cols],
                                     skip_group_check=True)
                    nc.tensor.matmul(zB[:], w_gob_sb[:], RT[g][:, cols],
                                     skip_group_check=True)

                    # all-tanh gates: TA = [ti @0 ; tf @64], TB = [tg @0 ; to @64]
                    TA = wpool.tile([128, GN], fp32, tag=f"TA{g}")
                    TB = wpool.tile([128, GN], fp32, tag=f"TB{g}")
                    nc.scalar.activation(TA[:], zA[:], Tanh)
                    nc.scalar.activation(TB[:], zB[:], Tanh)
                    TAhist[g].append(TA)
                    # DVE carrier: absorb the PE tick (covers ring WAR for H2)
                    nc.vector.tensor_copy(jnk[g][0:1, 0:1], zB[0:1, 0:1])

                    # D' = 0.5*(1+tf)*D + (1+ti)*tg      (state D = 2c @64)
                    Bt = wpool.tile([H, GN], fp32, tag=f"Bt{g}")
                    At = wpool.tile([H, GN], fp32, tag=f"At{g}")
                    nc.vector.scalar_tensor_tensor(
                        Bt[:], TA[64 : 64 + H, :], 1.0,
                        Dst[g][64 : 64 + H, :], add, mult)
                    nc.vector.scalar_tensor_tensor(
                        At[:], TA[0:H, :], 1.0, TB[0:H, :], add, mult)
                    nc.vector.scalar_tensor_tensor(
                        Dst[g][64 : 64 + H, :], Bt[:], 0.5, At[:], mult, add)

                    # H' = (1+to) * tanh(0.5*D') -> ring slot t+1, rows 64:114
                    TD = wpool.tile([128, GN], fp32, tag=f"TD{g}")
                    nc.scalar.activation(TD[64 : 64 + H, :],
                                         Dst[g][64 : 64 + H, :], Tanh,
                                         scale=0.5)
                    nc.vector.scalar_tensor_tensor(
                        RT[g][64 : 64 + H, ncols], TB[64 : 64 + H, :], 1.0,
                        TD[64 : 64 + H, :], add, mult)

            # final FC + sigmoid; H_last lives in slot S%RB (= 0)
            fsl = S % RB
            for g in range(G):
                fcols = slice(fsl * GN, (fsl + 1) * GN)
                po = ppool.tile([128, GN], fp32, tag=f"zA{g}", name="po")
                ta_old = TAhist[g][-2]
                nc.tensor.matmul(po[0:1, 0:1], ta_old[0:1, 0:1],
                                 ta_old[0:1, 0:1], skip_group_check=True)
                nc.tensor.matmul(po[0:1, :], w_fc_sb[:], RT[g][:, fcols],
                                 skip_group_check=True)
                to_sb = wpool.tile([1, GN], fp32, tag=f"to{g}")
                # sigmoid(u) = 0.5 + 0.5*tanh(0.5*u); b_fc folded into w_fc
                nc.scalar.activation(to_sb[:], po[0:1, :], Tanh, scale=0.5)
                o_sb = wpool.tile([1, GN], fp32, tag=f"o{g}")
                nc.vector.tensor_scalar(o_sb[:], to_sb[:], 0.5, 0.5, mult, add)
                nc.gpsimd.dma_start(out_d[0:1, g * GN : (g + 1) * GN], o_sb[:])

    return nc


def _prep_global(x, W_ih, W_hh, b_ih, b_hh, W_fc, b_fc):
    """Host-side weight preprocessing; returns {name: array}.

    xn is the global sharded array (axis 0 = 8 core shards of [BS, S]);
    wpack is a single replicated array.
    """
    x = np.asarray(x, np.float32)
    W_ih = np.asarray(W_ih, np.float32)
    W_hh = np.asarray(W_hh, np.float32)
    b = np.asarray(b_ih, np.float32) + np.asarray(b_hh, np.float32)
    W_fc = np.asarray(W_fc, np.float32)

    # gate rows: i(0:50) f(50:100) g(100:150) o(150:200)
    row_scale = np.full(4 * H, 0.5, np.float32)
    row_scale[2 * H : 3 * H] = 1.0  # g rows use tanh directly
    W_hh_eff = (row_scale[:, None] * W_hh * 0.5).astype(np.float32)  # H=2h comp
    W_ih_eff = (row_scale * W_ih[:, 0]).astype(np.float32)
    b_eff = (row_scale * b).astype(np.float32)

    # stationary weights [KK, 128]: row 0 = bias (vs ones), row 1 = x
    # weights, rows 64:114 = W_hh^T ; gate pair at cols 0:50 and 64:114
    def bank(g1, g2):
        w = np.zeros((KK, 128), np.float32)
        for col, lo in ((0, g1), (64, g2)):
            w[0, col : col + H] = b_eff[lo : lo + H]
            w[1, col : col + H] = W_ih_eff[lo : lo + H]
            w[64 : 64 + H, col : col + H] = W_hh_eff[lo : lo + H].T
        return w

    wpack = np.zeros((342, 128), np.float32)
    wpack[0:KK] = bank(0, H)                   # i cols 0:50, f cols 64:114
    wpack[KK : 2 * KK] = bank(2 * H, 3 * H)    # g cols 0:50, o cols 64:114
    # w_fc block: row 0 = b_fc (vs ones), rows 64:114 = 0.5*W_fc
    wpack[2 * KK, 0] = float(np.asarray(b_fc, np.float32).reshape(-1)[0])
    wpack[2 * KK + 64 : 2 * KK + 64 + H, 0] = 0.5 * W_fc[0, :]

    # x natural layout [B, S] fp16; axis 0 shards into 8 x [BS, S]
    x2 = x.reshape(B, S)
    xn_g = np.empty((B, S), np.float16)
    try:
        from concurrent.futures import ThreadPoolExecutor
        nth = 8
        rows = B // nth
        with ThreadPoolExecutor(nth) as pool:
            list(pool.map(
                lambda i: np.copyto(xn_g[i * rows:(i + 1) * rows],
                                    x2[i * rows:(i + 1) * rows],
                                    casting="same_kind"),
                range(nth)))
    except Exception:
        xn_g = x2.astype(np.float16)

    return {"xn": xn_g, "wpack": wpack}


def _get_runner():
    """Build (once) and cache the jitted shard_map executable."""
    if "runner" in _cache:
        return _cache["runner"]

    import jax
    from jax.sharding import Mesh, PartitionSpec
    from jax.experimental.shard_map import shard_map
    from concourse import bass2jax
    import concourse.mybir as mybir

    nc = _build()
    bass2jax.install_neuronx_cc_hook()
    partition_name = (nc.partition_id_tensor.name
                      if nc.partition_id_tensor else None)

    in_names, out_names, out_avals, zero_outs = [], [], [], []
    for alloc in nc.m.functions[0].allocations:
        if not isinstance(alloc, mybir.MemoryLocationSet):
            continue
        name = alloc.memorylocations[0].name
        if alloc.kind == "ExternalInput":
            if name != partition_name:
                in_names.append(name)
        elif alloc.kind == "ExternalOutput":
            out_names.append(name)
            shape = tuple(alloc.tensor_shape)
            dtype = mybir.dt.np(alloc.dtype)
            out_avals.append(jax.core.ShapedArray(shape, dtype))
            zero_outs.append(np.zeros(shape, dtype))
    n_params = len(in_names)
    n_outs = len(out_avals)
    in_names_all = list(in_names) + out_names
    if partition_name is not None:
        in_names_all.append(partition_name)
    donate = tuple(range(n_params, n_params + n_outs))

    def _body(*args):
        operands = list(args)
        if partition_name is not None:
            operands.append(bass2jax.partition_id_tensor())
        outs = bass2jax._bass_exec_p.bind(
            *operands,
            out_avals=tuple(out_avals),
            in_names=tuple(in_names_all),
            out_names=tuple(out_names),
            lowering_input_output_aliases=(),
            sim_require_finite=True,
            sim_require_nnan=True,
            nc=nc,
        )
        return tuple(outs)

    devices = jax.devices()[:NCORES]
    assert len(devices) == NCORES, (
        f"need {NCORES} devices, got {len(jax.devices())}")
    mesh = Mesh(np.asarray(devices), ("core",))
    # x sharded by core; small weight pack replicated
    spec_by_name = {"xn": PartitionSpec("core")}
    in_specs = tuple(
        [spec_by_name.get(n, PartitionSpec()) for n in in_names]
        + [PartitionSpec("core")] * n_outs)
    out_specs = (PartitionSpec("core"),) * len(out_names)
    sharded = jax.jit(
        shard_map(_body, mesh=mesh, in_specs=in_specs, out_specs=out_specs,
                  check_rep=False),
        donate_argnums=donate, keep_unused=True,
    )

    def run(global_in: dict):
        args = [global_in[name] for name in in_names]
        zeros = [np.zeros((NCORES * z.shape[0], *z.shape[1:]), z.dtype)
                 for z in zero_outs]
        out_arrs = sharded(*args, *zeros)
        # single output "out": global [NCORES, BS]
        return np.asarray(out_arrs[0])

    _cache["runner"] = run
    return run


def kernel(**inputs) -> np.ndarray:
    run = _get_runner()
    global_in = _prep_global(**inputs)
    out = run(global_in)
    return out.reshape(B, 1).astype(np.float32)
